# revision 16
# baseline (speedup 1.0000x reference)
"""Trainium2 Bass kernel for nn_AttentionBlock (GroupNorm + linear attention + proj + residual).

Full shapes: x [4, 256, 32, 32, 32] fp32, N = 32768 spatial positions.

Reference computation:
  norm = GroupNorm(4 groups)(x);  qkv = qkv_weight @ norm (1x1x1 conv)
  k = softmax(k, axis=spatial);  sim[h] = k[h] @ v[h].T  (hd x hd)
  out[h] = sim[h].T @ q[h];  out = out_weight @ out + out_bias + x

Sharding (8 cores): core c -> batch b = c//2, spatial half h2 = c%2.
Each core:
  - streams its x[b][:, half] as fp16 (host-cast, 8.4 MB) into SBUF while
    computing GN stats (sum rides the fp8-cast ACT pass; sumsq via a
    scalar_tensor_tensor accumulate on DVE)
  - per-tile pair AllReduce of channel sum/sumsq -> GN fold scales a,b
  - phase A: kv projection as ONE fp8 DoubleRow matmul per 128-position
    chunk (contraction 256 packed into the two DR slots).  Weight fp8
    quantization error is suppressed by error-feedback dithering: NSETS
    fp8 weight sets whose errors telescope; each set covers 1/NSETS of
    the positions, so the attention-averaged weight error is ~ulp/NSETS.
    exp(kT) and vT are written as fp8 (exp bias -2 keeps e4m3 range), and
    sim+denominator accumulate via fp8 DoubleRow matmuls pairing the two
    chunks.  Software-pipelined: sim of pair p-1 hides under kv of pair p.
  - AllReduce (pair) of sim partials
  - fold: W3 = a_c * (qw2.T @ sim_blockdiag @ owT) + I  (residual folded
    into the weight diagonal); ab/ob2 biases as rank-1 folds
  - phase B: out = (W3+I).T @ x + ob2 (fp16 matmuls, warmed-up PE), fp16
    DMA out; host upcasts to fp32

Algebraic tricks (validated vs reference + numpy error model):
  - GN fold: qkv(norm(x)) = (W * a_c) @ x + W @ b_c; a,b from group stats
  - k bias dropped entirely (softmax shift invariance)
  - softmax denominator = 32.0-column in the sim matmul rhs (cancels the
    32x fp8 scale of vT exactly)
  - v bias folded post-hoc: sim_norm = sim_raw/den + vbias (rank-1)
  - sim folded into q weights (skips materializing q entirely)
  - residual rides the phase-B matmul as +I on the folded weight matrix
  - fp8 scales (x*32, w*256) cancel via exp scale 2^-13 / vT scale 2^-8
"""
import numpy as np

import concourse.bass as bass
import concourse.bacc as bacc
import concourse.mybir as mybir
import concourse.tile as tile
from concourse import bass_utils

N_CORES = 8
B, C, Dd, Hh, Ww = 4, 256, 32, 32, 32
N = Dd * Hh * Ww           # 32768
NH = N // 2                # 16384 (per-core spatial half)
G = 4                      # groupnorm groups
EPS = 1e-5
f32 = mybir.dt.float32
f16 = mybir.dt.float16
f8 = mybir.dt.float8e4
AF = mybir.ActivationFunctionType
ALU = mybir.AluOpType
AX = mybir.AxisListType
DR = mybir.MatmulPerfMode.DoubleRow

REPLICA_GROUPS = [[0, 1], [2, 3], [4, 5], [6, 7]]

SX = 32.0     # fp8 scale for x
SW = 256.0    # fp8 scale for folded kv weights
SINV = 1.0 / (SX * SW)   # 2^-13
SV = 1.0 / 256.0         # vT copy scale: 2^-13 * 32 (vT = 32x v)
NSETS = 4     # dithered fp8 weight sets (error feedback)

# wpack column offsets (fp32 [128, WCOLS])
O_KVW = 0          # 2 x 512
O_QW = 1024        # 2 x 256 (qkv_weight[0:C].T tiles)
O_QW2 = 1536       # 2 x 256 (qkv_weight[0:C] row-major tiles)
O_OW = 2048        # 2 x 256 (out_weight.T tiles)
O_I256 = 2560      # 2 x 256 identity blocks
O_MASK = 3072      # 128 (head block-diag mask)
O_GNW = 3200       # 2 x 1
O_GNB = 3202       # 2 x 1
O_OB = 3204        # 2 x 1
O_IND = 3206       # 2 x 4
WCOLS = 3214


def build(nh=NH):
    """Build + compile the SPMD program. nh parameterized for fast sim tests."""
    stats_ch = min(4096, nh)
    n_stats_ch = nh // stats_ch
    n_pair = nh // 256         # phase A: 2x128-col sub-chunks per iter
    n_blk = nh // 512          # phase B 512-col blocks
    inv_n = 1.0 / (64.0 * 2 * nh)   # group stats count: 64 ch x full N (=2*nh)
    set_pairs = max(1, n_pair // NSETS)

    nc = bacc.Bacc("TRN2", target_bir_lowering=False, debug=False,
                   num_devices=N_CORES)

    xh_d = nc.dram_tensor("xh", [2, 128, nh], f16, kind="ExternalInput")
    wp_d = nc.dram_tensor("wp", [128, WCOLS], f32, kind="ExternalInput")
    sp_d = nc.dram_tensor("sp", [4, 256], f32, kind="ExternalInput")
    out_d = nc.dram_tensor("out", [2, 128, nh], f16, kind="ExternalOutput")

    with tile.TileContext(nc) as tc:
        with tc.tile_pool(name="const", bufs=1) as cp, \
             tc.tile_pool(name="dram", bufs=1, space="DRAM") as dp:
            # ---- persistent SBUF tiles ----
            xc = [cp.tile([128, nh], f16, name=f"xc{t}", tag=f"xc{t}") for t in range(2)]
            xq = cp.tile([128, 2, nh], f8, name="xq", tag="xq")
            wt = cp.tile([128, WCOLS], f32, name="wt", tag="wt")
            spk = cp.tile([4, 256], f32, name="spk", tag="spk")
            kvq = [cp.tile([128, 2, 512], f8, name=f"kvq{j}", tag=f"kvq{j}")
                   for j in range(NSETS)]
            kvres = cp.tile([128, 2, 512], f32, name="kvres", tag="kvres")
            kvtgt = cp.tile([128, 2, 512], f32, name="kvtgt", tag="kvtgt")
            W3f = [cp.tile([128, 256], f16, name=f"W3f{t}", tag=f"W3f{t}") for t in range(2)]
            ab_col = [cp.tile([128, 1], f32, name=f"abc{t}", tag=f"abc{t}") for t in range(2)]
            ob2 = [cp.tile([128, 1], f32, name=f"ob2{t}", tag=f"ob2{t}") for t in range(2)]
            ones_row = cp.tile([1, 128], f32, name="ones_row", tag="ones_row")
            a_sb = [cp.tile([128, 1], f32, name=f"a{t}", tag=f"a{t}") for t in range(2)]
            a2_sb = [cp.tile([128, 1], f32, name=f"a2{t}", tag=f"a2{t}") for t in range(2)]
            b_sb = [cp.tile([128, 1], f32, name=f"b{t}", tag=f"b{t}") for t in range(2)]
            qb_sb = [cp.tile([128, 1], f32, name=f"qb{t}", tag=f"qb{t}") for t in range(2)]
            vb_sb = cp.tile([1, 256], f32, name="vb", tag="vb")
            vbb_sb = [cp.tile([128, 128], f32, name=f"vbb{t}", tag=f"vbb{t}") for t in range(2)]
            simbd = [cp.tile([128, 128], f32, name=f"simbd{t}", tag=f"simbd{t}") for t in range(2)]
            # phase A double-buffered vT tiles ([s2, dt, 128 v + 1 den-col])
            vt2 = [cp.tile([128, 2, 2, 129], f8, name=f"vt{i}", tag=f"vt{i}")
                   for i in range(2)]

            # weight views
            kvw = [wt[:, O_KVW + t * 512: O_KVW + (t + 1) * 512] for t in range(2)]
            qw = [wt[:, O_QW + t * 256: O_QW + (t + 1) * 256] for t in range(2)]
            qw2 = [wt[:, O_QW2 + t * 256: O_QW2 + (t + 1) * 256] for t in range(2)]
            owf = [wt[:, O_OW + t * 256: O_OW + (t + 1) * 256] for t in range(2)]
            I256 = [wt[:, O_I256 + t * 256: O_I256 + (t + 1) * 256] for t in range(2)]
            mask = wt[:, O_MASK: O_MASK + 128]
            gnw = [wt[:, O_GNW + t: O_GNW + t + 1] for t in range(2)]
            gnb = [wt[:, O_GNB + t: O_GNB + t + 1] for t in range(2)]
            obv = [wt[:, O_OB + t: O_OB + t + 1] for t in range(2)]
            ind = [wt[:, O_IND + t * 4: O_IND + (t + 1) * 4] for t in range(2)]
            indT = [spk[:, t * 128: (t + 1) * 128] for t in range(2)]

            nc.scalar.dma_start(wt[:], wp_d.ap())
            nc.scalar.dma_start(spk[:], sp_d.ap())
            ekb = cp.tile([128, 1], f32, name="ekb", tag="ekb")
            nc.vector.memset(ekb[:], -2.0)
            nc.vector.memset(ones_row[:], 1.0)
            for i in range(2):
                nc.vector.memset(vt2[i][:, :, :, 128:129], SX)

            # ---- x load (fp16, direct into cache) + streaming stats ----
            with tc.tile_pool(name="sp", bufs=1) as sp, \
                 tc.tile_pool(name="spp", bufs=1, space="PSUM") as spp:
                scol_s = [sp.tile([128, n_stats_ch], f32, name=f"scs{t}", tag=f"scs{t}") for t in range(2)]
                scol_q = [sp.tile([128, n_stats_ch], f32, name=f"scq{t}", tag=f"scq{t}") for t in range(2)]
                stat2 = [sp.tile([128, 2], f32, name=f"st{t}", tag=f"st{t}") for t in range(2)]
                stat2r = [sp.tile([128, 2], f32, name=f"str{t}", tag=f"str{t}") for t in range(2)]

                sa_in = [dp.tile([128, 2], f32, name=f"sa_in{t}", tag=f"sa_in{t}") for t in range(2)]
                sa_out = [dp.tile([128, 2], f32, name=f"sa_out{t}", tag=f"sa_out{t}") for t in range(2)]
                dml = sp.tile([1, 1], f32, name="dml", tag="dml")
                for t in range(2):
                    for i in range(n_stats_ch):
                        sl = slice(i * stats_ch, (i + 1) * stats_ch)
                        nc.sync.dma_start(xc[t][:, sl], xh_d.ap()[t, :, sl])
                        # sumsq on DVE: (x*1) * x with accumulate
                        dsq = sp.tile([128, stats_ch], f16, name="dsq", tag="dsq", bufs=2)
                        nc.vector.scalar_tensor_tensor(
                            dsq[:], xc[t][:, sl], 1.0, xc[t][:, sl],
                            op0=ALU.mult, op1=ALU.mult,
                            accum_out=scol_q[t][:, i:i + 1])
                        if t == 0:
                            # fp8 cast + 32*sum in one ACT pass
                            nc.scalar.activation(xq[:, 0, sl], xc[0][:, sl],
                                                 AF.Copy, scale=SX,
                                                 accum_out=scol_s[0][:, i:i + 1])
                        else:
                            # 32*sum on DVE (throwaway out, fast mode)
                            dsm = sp.tile([128, stats_ch], f16, name="dsm", tag="dsm", bufs=2)
                            nc.vector.tensor_scalar(
                                dsm[:], xc[1][:, sl], SX, None,
                                op0=ALU.mult, op1=ALU.add,
                                accum_out=scol_s[1][:, i:i + 1])
                        if i == 0 and t == 0:
                            # anchored dummy: force the ln/exp ACT table load early
                            nc.scalar.activation(dml[:], xc[0][0:1, 0:1], AF.Ln,
                                                 scale=0.0, bias=1.0)
                    nc.vector.reduce_sum(stat2[t][:, 0:1], scol_s[t][:], axis=AX.X)
                    nc.vector.reduce_sum(stat2[t][:, 1:2], scol_q[t][:], axis=AX.X)
                    # per-tile AllReduce: tile 0's round hides under tile 1's load
                    nc.sync.dma_start(sa_in[t][:], stat2[t][:])
                    nc.gpsimd.collective_compute(
                        "AllReduce", ALU.add, replica_groups=REPLICA_GROUPS,
                        ins=[sa_in[t][:].opt()], outs=[sa_out[t][:].opt()])
                # fp8 cast of tile 1 on ACT (hides under the stats AllReduce)
                nc.scalar.activation(xq[:, 1, :], xc[1][:], AF.Copy, scale=SX)
                for t in range(2):
                    nc.sync.dma_start(stat2r[t][:], sa_out[t][:])

                # group stats: [4,2] = indicator.T @ (32*sum|sumsq)
                gps = spp.tile([4, 2], f32, name="gps", tag="gps")
                for t in range(2):
                    nc.tensor.matmul(gps[:], ind[t][:], stat2r[t][:],
                                     start=(t == 0), stop=(t == 1))
                eps4 = sp.tile([4, 1], f32, name="eps4", tag="eps4")
                nc.vector.memset(eps4[:], EPS)
                msm = sp.tile([4, 1], f32, name="msm", tag="msm")
                vs = sp.tile([4, 1], f32, name="vs", tag="vs")
                msq = sp.tile([4, 1], f32, name="msq", tag="msq")
                var = sp.tile([4, 1], f32, name="var", tag="var")
                lnv = sp.tile([4, 1], f32, name="lnv", tag="lnv")
                rstd = sp.tile([4, 1], f32, name="rstd", tag="rstd")
                rm = sp.tile([4, 2], f32, name="rm", tag="rm")
                nc.vector.tensor_scalar_mul(msm[:], gps[:, 0:1], inv_n / SX)
                nc.vector.tensor_scalar_mul(vs[:], gps[:, 1:2], inv_n)
                nc.vector.tensor_mul(msq[:], msm[:], msm[:])
                nc.vector.tensor_sub(var[:], vs[:], msq[:])
                nc.scalar.activation(lnv[:], var[:], AF.Ln, bias=eps4[:])
                nc.scalar.activation(rstd[:], lnv[:], AF.Exp, scale=-0.5)
                nc.vector.tensor_copy(rm[:, 0:1], rstd[:])
                nc.vector.tensor_copy(rm[:, 1:2], msm[:])

                # broadcast to per-channel: chan[t] = indT.T @ (rstd|mean)
                ma = [sp.tile([128, 1], f32, name=f"ma{t}", tag=f"ma{t}") for t in range(2)]
                for t in range(2):
                    chan = spp.tile([128, 2], f32, name=f"chan{t}", tag=f"chan{t}")
                    nc.tensor.matmul(chan[:], indT[t], rm[:])
                    nc.vector.tensor_mul(a_sb[t][:], chan[:, 0:1], gnw[t])
                    nc.vector.tensor_mul(ma[t][:], chan[:, 1:2], a_sb[t][:])
                    nc.vector.tensor_sub(b_sb[t][:], gnb[t], ma[t][:])
                    nc.vector.tensor_scalar_mul(a2_sb[t][:], a_sb[t][:], SW)
                    # dither set 0: plain fp8 quantize of folded weights
                    nc.vector.tensor_scalar_mul(kvq[0][:, t, :], kvw[t], a2_sb[t][:])
                    # its residual (error feedback seed)
                    nc.vector.scalar_tensor_tensor(
                        kvres[:, t, :], kvw[t], a2_sb[t][:], kvq[0][:, t, :],
                        op0=ALU.mult, op1=ALU.subtract)

                # q bias: qb[dt] = qwT.T @ b_fold   (unscaled qw)
                for dt in range(2):
                    qb_ps = spp.tile([128, 1], f32, name=f"qbp{dt}", tag=f"qbp{dt}")
                    for t in range(2):
                        nc.tensor.matmul(qb_ps[:], qw[t][:, dt * 128:(dt + 1) * 128],
                                         b_sb[t][:], start=(t == 0), stop=(t == 1))
                    nc.vector.tensor_copy(qb_sb[dt][:], qb_ps[:])
                # v bias row: vb = b_fold.T @ vwT
                vb_ps = spp.tile([1, 256], f32, name="vbp", tag="vbp")
                for t in range(2):
                    nc.tensor.matmul(vb_ps[:], b_sb[t][:], kvw[t][:, 256:512],
                                     start=(t == 0), stop=(t == 1))
                nc.vector.tensor_copy(vb_sb[:], vb_ps[:])
                # broadcast vbias rows across partitions (rank-1 with ones)
                for dt in range(2):
                    vbb_ps = spp.tile([128, 128], f32, name=f"vbbp{dt}", tag=f"vbbp{dt}")
                    nc.tensor.matmul(vbb_ps[:], ones_row[:],
                                     vb_sb[:, dt * 128:(dt + 1) * 128])
                    nc.vector.tensor_copy(vbb_sb[dt][:], vbb_ps[:])

            def gen_set(j, last):
                """Emit dither set j from the running residual (error feedback)."""
                for t in range(2):
                    nc.vector.scalar_tensor_tensor(
                        kvtgt[:, t, :], kvw[t], a2_sb[t][:], kvres[:, t, :],
                        op0=ALU.mult, op1=ALU.add)
                nc.scalar.activation(kvq[j][:], kvtgt[:], AF.Copy)
                if not last:
                    nc.vector.tensor_sub(kvres[:], kvtgt[:], kvq[j][:])

            # ---- phase A: fp8 DR kv matmuls + fp8 DR sim accumulation ----
            # software pipelined: sim matmuls of pair p-1 issue after kv of pair p
            with tc.tile_pool(name="pa", bufs=1) as pa, \
                 tc.tile_pool(name="pap", bufs=1, space="PSUM") as pap:
                if NSETS > 1:
                    gen_set(1, NSETS == 2)
                sim_ps = [pap.tile([128, 129], f32, name=f"sim{dt}", tag=f"sim{dt}") for dt in range(2)]
                ek_prev = None
                vt_prev = None

                def sim_mms(p, ek, vtb):
                    first, last = (p == 0), (p == n_pair - 1)
                    for dt in range(2):
                        nc.tensor.matmul(
                            sim_ps[dt][:],
                            ek[:, :, dt * 128:(dt + 1) * 128],
                            vtb[:, :, dt, :],
                            perf_mode=DR, start=first, stop=last)

                for p in range(n_pair):
                    jset = min(p // set_pairs, NSETS - 1)
                    kv_ps = pap.tile([128, 1024], f32, name="kv", tag="kv", bufs=2)
                    for s2 in range(2):
                        s = 2 * p + s2
                        nc.tensor.matmul(kv_ps[:, s2 * 512:(s2 + 1) * 512],
                                         xq[:, :, s * 128:(s + 1) * 128],
                                         kvq[jset][:], perf_mode=DR)
                    if ek_prev is not None:
                        sim_mms(p - 1, ek_prev, vt_prev)
                    ek = pa.tile([128, 2, 256], f8, name="ek", tag="ek", bufs=2)
                    kv_k = kv_ps[:].rearrange("p (s d) -> p s d", s=2)[:, :, 0:256]
                    nc.scalar.activation(ek[:], kv_k, AF.Exp, scale=SINV,
                                         bias=ekb[:])
                    vtb = vt2[p % 2]
                    kv_v = kv_ps[:].rearrange("p (s d c) -> p s d c", s=2, d=4)[:, :, 2:4, :]
                    nc.vector.tensor_scalar_mul(vtb[:, :, :, 0:128], kv_v, SV)
                    ek_prev, vt_prev = ek, vtb
                    # emit later dither sets early in the loop (engine slack),
                    # always before their first use at pair j*set_pairs
                    if NSETS > 2 and p == (2 if set_pairs > 2 else 0):
                        gen_set(2, False)
                    if NSETS > 3 and p == (set_pairs // 2 if set_pairs > 2 else 1):
                        gen_set(3, True)
                sim_mms(n_pair - 1, ek_prev, vt_prev)

                # pair AllReduce of sim partials (+denominator column)
                sim_sb = [pa.tile([128, 129], f32, name=f"simsb{dt}", tag=f"simsb{dt}") for dt in range(2)]
                simr = [pa.tile([128, 129], f32, name=f"simr{dt}", tag=f"simr{dt}") for dt in range(2)]
                si_in = dp.tile([2, 128, 129], f32, name="si_in", tag="si_in")
                si_out = dp.tile([2, 128, 129], f32, name="si_out", tag="si_out")
                for dt in range(2):
                    nc.vector.tensor_copy(sim_sb[dt][:], sim_ps[dt][:])
                    nc.sync.dma_start(si_in[dt], sim_sb[dt][:])
                nc.gpsimd.collective_compute(
                    "AllReduce", ALU.add, replica_groups=REPLICA_GROUPS,
                    ins=[si_in[:].opt()], outs=[si_out[:].opt()])
                for dt in range(2):
                    nc.sync.dma_start(simr[dt][:], si_out[dt])

                # warm-up matmuls anchored on the AllReduce result: ~3.4us of
                # sustained PE activity flips the clock gate to 8/8 so the
                # fold + phase B matmul stream runs at full rate
                warm = pap.tile([128, 512], f32, name="warm", tag="warm")
                for wi in range(3):
                    nc.tensor.matmul(warm[:], simr[0][:, 0:128], wt[:, 0:512],
                                     start=True, stop=True, skip_group_check=True)

                # normalize + vbias + block-diag mask
                for dt in range(2):
                    recip = pa.tile([128, 1], f32, name=f"rec{dt}", tag=f"rec{dt}")
                    simn = pa.tile([128, 128], f32, name=f"simn{dt}", tag=f"simn{dt}")
                    nc.vector.reciprocal(recip[:], simr[dt][:, 128:129])
                    nc.vector.scalar_tensor_tensor(
                        simn[:], simr[dt][:, 0:128], recip[:], vbb_sb[dt][:],
                        op0=ALU.mult, op1=ALU.add)
                    nc.vector.tensor_mul(simbd[dt][:], simn[:], mask)

            # ---- fold sim+proj+residual into one matrix: out = W3.T@x + ob2 ----
            # W2rawT[et] = simbd[et].T @ qw2[et]   ([e, c])
            # W3[ct] = a_c * sum_et W2rawT[et][:, ct].T @ owT[et] + I  ([c, o])
            # ob2[ot] = sum_et owT[et][:, ot].T @ (simbd[et].T @ qb[et]) + out_bias
            with tc.tile_pool(name="pwsb", bufs=1) as pwsb, \
                 tc.tile_pool(name="pw", bufs=1, space="PSUM") as pw:
                w2rt = [pwsb.tile([128, 256], f32, name=f"w2rt{et}", tag=f"w2rt{et}")
                        for et in range(2)]
                for et in range(2):
                    w2_ps = pw.tile([128, 256], f32, name=f"w2p{et}", tag=f"w2p{et}")
                    nc.tensor.matmul(w2_ps[:], simbd[et][:], qw2[et])
                    nc.vector.tensor_copy(w2rt[et][:], w2_ps[:])
                for ct in range(2):
                    w3_ps = pw.tile([128, 256], f32, name=f"w3p{ct}", tag=f"w3p{ct}")
                    for et in range(2):
                        nc.tensor.matmul(w3_ps[:], w2rt[et][:, ct * 128:(ct + 1) * 128],
                                         owf[et], start=(et == 0), stop=(et == 1))
                    nc.vector.scalar_tensor_tensor(
                        W3f[ct][:], w3_ps[:], a_sb[ct][:], I256[ct],
                        op0=ALU.mult, op1=ALU.add)
                for et in range(2):
                    ab_ps = pw.tile([128, 1], f32, name=f"abp{et}", tag=f"abp{et}")
                    nc.tensor.matmul(ab_ps[:], simbd[et][:], qb_sb[et][:])
                    nc.vector.tensor_copy(ab_col[et][:], ab_ps[:])
                for ot in range(2):
                    ob2_ps = pw.tile([128, 1], f32, name=f"ob2p{ot}", tag=f"ob2p{ot}")
                    for et in range(2):
                        nc.tensor.matmul(ob2_ps[:], owf[et][:, ot * 128:(ot + 1) * 128],
                                         ab_col[et][:], start=(et == 0), stop=(et == 1))
                    nc.vector.tensor_add(ob2[ot][:], ob2_ps[:], obv[ot])

            # ---- phase B: out = (W3+I).T@x + ob2 (bias+residual included) ----
            with tc.tile_pool(name="pb", bufs=1) as pb, \
                 tc.tile_pool(name="pbp", bufs=4, space="PSUM") as pbp:
                ob_blk = min(4, n_blk)
                for sup in range(n_blk // ob_blk):
                    obig = [pb.tile([128, ob_blk * 512], f16, name=f"os{ot}", tag=f"os{ot}",
                                    bufs=2) for ot in range(2)]
                    for sub in range(ob_blk):
                        blk = sup * ob_blk + sub
                        sl = slice(blk * 512, (blk + 1) * 512)
                        so = slice(sub * 512, (sub + 1) * 512)
                        for ot in range(2):
                            pr_ps = pbp.tile([128, 512], f32, name=f"mm{ot}", tag=f"mm{ot}")
                            nc.tensor.matmul(pr_ps[:], W3f[0][:, ot * 128:(ot + 1) * 128],
                                             xc[0][:, sl], start=True, stop=False)
                            nc.tensor.matmul(pr_ps[:], W3f[1][:, ot * 128:(ot + 1) * 128],
                                             xc[1][:, sl], start=False, stop=True)
                            if ot == 0:
                                nc.scalar.activation(obig[ot][:, so], pr_ps[:],
                                                     AF.Identity, bias=ob2[ot][:])
                            else:
                                nc.vector.tensor_scalar_add(obig[ot][:, so], pr_ps[:],
                                                            ob2[ot][:])
                    for ot in range(2):
                        nc.sync.dma_start(
                            out_d.ap()[ot, :, sup * ob_blk * 512:(sup + 1) * ob_blk * 512],
                            obig[ot][:])

    nc.compile()
    return nc


_NC = None


def _get_nc():
    global _NC
    if _NC is None:
        _NC = build()
    return _NC


def make_wpack(gn_weight, gn_bias, qkv_weight, out_weight, out_bias):
    qkv_weight = np.asarray(qkv_weight, dtype=np.float32)
    out_weight = np.asarray(out_weight, dtype=np.float32)
    wp = np.zeros((128, WCOLS), np.float32)
    kvwT = np.ascontiguousarray(
        np.concatenate([qkv_weight[C:2 * C], qkv_weight[2 * C:3 * C]], axis=0).T
    ).reshape(2, 128, 512)
    wp[:, O_KVW:O_KVW + 1024] = np.concatenate([kvwT[0], kvwT[1]], axis=1)
    qwT = np.ascontiguousarray(qkv_weight[0:C].T).reshape(2, 128, 256)
    wp[:, O_QW:O_QW + 512] = np.concatenate([qwT[0], qwT[1]], axis=1)
    qw2 = np.ascontiguousarray(qkv_weight[0:C]).reshape(2, 128, 256)
    wp[:, O_QW2:O_QW2 + 512] = np.concatenate([qw2[0], qw2[1]], axis=1)
    owT = np.ascontiguousarray(out_weight.T).reshape(2, 128, 256)
    wp[:, O_OW:O_OW + 512] = np.concatenate([owT[0], owT[1]], axis=1)
    eye = np.eye(256, dtype=np.float32).reshape(2, 128, 256)
    wp[:, O_I256:O_I256 + 512] = np.concatenate([eye[0], eye[1]], axis=1)
    mask = np.zeros((128, 128), np.float32)
    for h in range(4):
        mask[h * 32:(h + 1) * 32, h * 32:(h + 1) * 32] = 1.0
    wp[:, O_MASK:O_MASK + 128] = mask
    wp[:, O_GNW:O_GNW + 2] = np.asarray(gn_weight, np.float32).reshape(2, 128).T
    wp[:, O_GNB:O_GNB + 2] = np.asarray(gn_bias, np.float32).reshape(2, 128).T
    wp[:, O_OB:O_OB + 2] = np.asarray(out_bias, np.float32).reshape(2, 128).T
    indf = np.zeros((C, G), np.float32)
    indf[np.arange(C), np.arange(C) // 64] = 1.0
    ind2 = indf.reshape(2, 128, 4)
    wp[:, O_IND:O_IND + 8] = np.concatenate([ind2[0], ind2[1]], axis=1)
    indT = np.ascontiguousarray(indf.T)            # [4, 256]
    spk = np.concatenate([indT[:, 0:128], indT[:, 128:256]], axis=1).copy()
    return wp, spk


def make_in_maps(x, gn_weight, gn_bias, qkv_weight, out_weight, out_bias, nh=NH):
    x = np.asarray(x)
    n = 2 * nh
    wp, spk = make_wpack(gn_weight, gn_bias, qkv_weight, out_weight, out_bias)
    shared = {"wp": wp, "sp": spk}
    in_maps = []
    for c in range(N_CORES):
        b, h2 = c // 2, c % 2
        xb = x[b].reshape(C, n)
        xh = np.ascontiguousarray(
            xb[:, h2 * nh:(h2 + 1) * nh].astype(np.float16)).reshape(2, 128, nh)
        in_maps.append({"xh": xh, **shared})
    return in_maps


def assemble(results, nh=NH):
    n = 2 * nh
    out = np.empty((B, C, n), np.float32)
    for c in range(N_CORES):
        b, h2 = c // 2, c % 2
        out[b][:, h2 * nh:(h2 + 1) * nh] = results[c]["out"].reshape(C, nh).astype(np.float32)
    return out


def kernel(x, gn_weight, gn_bias, qkv_weight, out_weight, out_bias):
    nc = _get_nc()
    in_maps = make_in_maps(x, gn_weight, gn_bias, qkv_weight, out_weight, out_bias)
    last_err = None
    for _attempt in range(3):
        try:
            res = bass_utils.run_bass_kernel_spmd(
                nc, in_maps, core_ids=list(range(N_CORES)))
            break
        except Exception as e:  # transient NRT device errors recover on retry
            last_err = e
    else:
        raise last_err
    return assemble(res.results).reshape(B, C, Dd, Hh, Ww)
